# revision 11
# baseline (speedup 1.0000x reference)
"""Trainium2 Bass kernel for the GNN edge/triplet MLP problem.

Strategy (8 NeuronCores, SPMD, edge/triplet-parallel, no collectives):
  - Node features for each edge/triplet endpoint are gathered host-side during
    input marshalling (this container's bedrock image has no working
    device-side gather primitive: the dma_gather ext-isa ucode is absent and
    dynamic indirect DMA measures ~660ns/row).  The gathered features stream
    sequentially HBM->SBUF in bf16, already transposed to the
    features-on-partitions layout the TensorEngine wants.
  - The RBF expansion exp(-10*(mu_m - r)^2) is re-expressed in a tanh
    translate basis fitted host-side: rbf = T @ C^T with
    T[e,j] = tanh(alpha*(g_j - r_e)).  C^T folds into the first-layer weights
    (the layer is linear), so the device evaluates one ScalarE tanh per
    edge-block (scale and per-partition bias are free on ACT) -- no exp, so
    ScalarE never switches activation-table sets (tanh and silu share a set),
    and the basis uses all 128 partitions (full K=128 matmul chunks).
  - 4-layer MLP in bf16, H-major activations [256=2x128, B] so every layer is
    plain K-chunk accumulated matmuls into PSUM; SiLU on ScalarE evicts
    PSUM->SBUF(bf16).  The final [256->3] layer packs 4 blocks of 512 edges
    into one PSUM bank at partition offsets {0,32,64,96} via tile_position, so
    the bias-add eviction is one cheap [128,512] VectorE op.
  - Outputs are written as [3, n_shard] and transposed host-side.
"""

import os
import sys

import numpy as np
import ml_dtypes

for _p in ("/opt/trn_rl_repo",):
    if _p not in sys.path and os.path.isdir(_p):
        sys.path.append(_p)

import concourse.bass as bass
import concourse.tile as tile
from concourse import bacc, mybir
from concourse.bass_utils import run_bass_kernel_spmd

BF16 = mybir.dt.bfloat16
F32 = mybir.dt.float32
AF = mybir.ActivationFunctionType

N_CORES = 8
N, F, H, E, T, OUT = 20000, 128, 256, 262144, 393216, 3
E_SH, T_SH = E // N_CORES, T // N_CORES
SB = 2048  # edges per super-block (PSUM half-region = [128, 2048] f32 = 4 banks)
ALPHA = 8.0
STEP = 0.1
GRID128_LO, GRID126_LO = -1.0, -0.9


def _tanh_grid(npts, lo):
    return lo + STEP * np.arange(npts, dtype=np.float64)


def _fit_C(grid, mu, alpha=ALPHA, reg=1e-6):
    """Least-squares C s.t. exp(-10(mu_m - r)^2) ~= sum_j C[m,j] tanh(alpha (g_j - r))."""
    r = np.linspace(-0.5, 10.5, 8001)
    Tm = np.tanh(alpha * (grid[:, None] - r[None, :]))
    G = np.exp((-1.0 / STEP) * (mu[:, None].astype(np.float64) - r[None, :]) ** 2)
    A = Tm @ Tm.T + reg * np.eye(len(grid))
    return np.linalg.solve(A, Tm @ G.T).T  # [len(mu), len(grid)]


def _bf16(x):
    return np.asarray(x, dtype=np.float32).astype(ml_dtypes.bfloat16)


class _Phase:
    def __init__(self, name, n_sh, g_names, rbf_specs, w0_chunks,
                 w1_name, w2_name, w3_name, b3_name, out_name):
        self.name = name
        self.n_sh = n_sh
        self.n_sup = n_sh // SB
        self.g_names = g_names          # pre-gathered feature tensors [n_sup,128,SB]
        self.rbf_specs = rbf_specs      # (r_name, bias_key, n_rows, has_cs)
        self.w0_chunks = w0_chunks
        self.w1_name, self.w2_name, self.w3_name = w1_name, w2_name, w3_name
        self.b3_name = b3_name
        self.out_name = out_name


def build_program(e_sh=E_SH, t_sh=T_SH):
    assert e_sh % SB == 0 and t_sh % SB == 0
    nc = bacc.Bacc("TRN2", target_bir_lowering=False, debug=False,
                   num_devices=N_CORES, num_swdge_queues=4)

    dram = {}

    def din(name, shape, dt):
        dram[name] = nc.dram_tensor(name, list(shape), dt, kind="ExternalInput").ap()

    def dout(name, shape, dt):
        dram[name] = nc.dram_tensor(name, list(shape), dt, kind="ExternalOutput").ap()

    din("gsrc", (e_sh // SB, 128, SB), BF16)
    din("gdst", (e_sh // SB, 128, SB), BF16)
    din("gts", (t_sh // SB, 128, SB), BF16)
    din("gtdi", (t_sh // SB, 128, SB), BF16)
    din("gtdj", (t_sh // SB, 128, SB), BF16)
    din("enorm", (e_sh,), F32)
    din("nij", (t_sh,), F32)
    din("nik", (t_sh,), F32)
    din("cs16", (2, t_sh), BF16)
    din("eW0p", (3, 128, H), BF16)
    din("eW1p", (2, 128, H), BF16)
    din("eW2p", (2, 128, H), BF16)
    din("eW3p", (2, 128, 32), BF16)
    din("tW0p", (5, 128, H), BF16)
    din("tW1p", (2, 128, H), BF16)
    din("tW2p", (2, 128, H), BF16)
    din("tW3p", (2, 128, 32), BF16)
    din("gb128", (128, 1), F32)
    din("gb126", (128, 1), F32)
    din("eb3col", (128, 1), F32)
    din("tb3col", (128, 1), F32)
    dout("out_e3", (OUT, e_sh), F32)
    dout("out_t3", (OUT, t_sh), F32)

    edges = _Phase("e", e_sh, ["gsrc", "gdst"],
                   [("enorm", "gb128", 128, False)], 3,
                   "eW1p", "eW2p", "eW3p", "eb3col", "out_e3")
    trips = _Phase("t", t_sh, ["gts", "gtdi", "gtdj"],
                   [("nij", "gb126", 126, True), ("nik", "gb128", 128, False)], 5,
                   "tW1p", "tW2p", "tW3p", "tb3col", "out_t3")

    with tile.TileContext(nc) as tc:
        with tc.tile_pool(name="weights", bufs=1) as wpool, \
             tc.tile_pool(name="consts", bufs=1) as cpool:
            wt = {}
            for wname, nch, ncol in (("eW0p", 3, H), ("eW1p", 2, H), ("eW2p", 2, H),
                                     ("eW3p", 2, 32), ("tW0p", 5, H), ("tW1p", 2, H),
                                     ("tW2p", 2, H), ("tW3p", 2, 32)):
                tiles = []
                for c in range(nch):
                    wtile = wpool.tile([128, ncol], BF16, tag=f"{wname}{c}")
                    nc.sync.dma_start(out=wtile[:], in_=dram[wname][c])
                    tiles.append(wtile)
                wt[wname] = tiles
            ct = {}
            for cname in ("gb128", "gb126", "eb3col", "tb3col"):
                ctile = cpool.tile([128, 1], F32, tag=cname)
                nc.sync.dma_start(out=ctile[:], in_=dram[cname])
                ct[cname] = ctile

            for ph in (edges, trips):
                _emit_phase(tc, nc, dram, wt, ct, ph)

    nc.compile()
    return nc


def _emit_phase(tc, nc, dram, wt, ct, ph):
    w0 = wt[f"{ph.name}W0p"]
    w1, w2, w3 = wt[ph.w1_name], wt[ph.w2_name], wt[ph.w3_name]
    b3col = ct[ph.b3_name]
    out3 = dram[ph.out_name]

    with tc.tile_pool(name=f"{ph.name}_g", bufs=2) as gpool, \
         tc.tile_pool(name=f"{ph.name}_rb", bufs=2) as rbpool, \
         tc.tile_pool(name=f"{ph.name}_T", bufs=2) as tpool, \
         tc.tile_pool(name=f"{ph.name}_act", bufs=2) as actpool, \
         tc.tile_pool(name=f"{ph.name}_out", bufs=3) as outpool, \
         tc.tile_pool(name=f"{ph.name}_ps", bufs=2, space="PSUM") as pspool:
        for sb in range(ph.n_sup):
            sl = slice(sb * SB, (sb + 1) * SB)

            # --- pre-gathered feature tiles [128 feats, SB] bf16 ---
            rhs_tiles = []
            for li, gname in enumerate(ph.g_names):
                g = gpool.tile([128, SB], BF16, tag=f"g{li}")
                nc.gpsimd.dma_start(out=g[:], in_=dram[gname][sb])
                rhs_tiles.append(g[:])

            # --- tanh-basis RBF tiles ---
            for ri, (rname, gbkey, nrow, has_cs) in enumerate(ph.rbf_specs):
                rb = rbpool.tile([128, SB], F32, tag=f"rb{ri}")
                nc.sync.dma_start(
                    out=rb[:nrow, :],
                    in_=dram[rname][sl].partition_broadcast(nrow))
                tt = tpool.tile([128, SB], BF16, tag=f"T{ri}")
                nc.scalar.activation(
                    out=tt[:nrow, :], in_=rb[:nrow, :], func=AF.Tanh,
                    bias=ct[gbkey][:nrow, :], scale=-ALPHA)
                if has_cs:
                    nc.sync.dma_start(out=tt[126:128, :], in_=dram["cs16"][:, sl])
                rhs_tiles.append(tt[:])

            # --- layers 1..3: K-chunk matmuls into PSUM halves + SiLU ---
            acts = rhs_tiles
            for layer, wtiles in ((1, w0), (2, w1), (3, w2)):
                new_acts = []
                for half in (0, 1):
                    ps = pspool.tile([128, SB], F32, tag="mm")
                    hsl = slice(128 * half, 128 * half + 128)
                    for c in range(SB // 512):
                        csl = slice(512 * c, 512 * (c + 1))
                        for k, (wtile, rhs) in enumerate(zip(wtiles, acts)):
                            nc.tensor.matmul(
                                ps[:, csl], wtile[:, hsl], rhs[:, csl],
                                start=(k == 0), stop=(k == len(wtiles) - 1))
                    a = actpool.tile([128, SB], BF16, tag=f"a{layer}h{half}")
                    nc.scalar.activation(out=a[:], in_=ps[:], func=AF.Silu)
                    new_acts.append(a[:])
                acts = new_acts

            # --- layer 4: [256 -> 3], packed 4x512 into one PSUM bank ---
            ps4 = pspool.tile([128, 512], F32, tag="mm")
            for g4 in range(4):
                gsl = slice(512 * g4, 512 * (g4 + 1))
                for half in (0, 1):
                    nc.tensor.matmul(
                        ps4[32 * g4:32 * g4 + 32, :], w3[half][:, 0:32],
                        acts[half][:, gsl], start=(half == 0), stop=(half == 1),
                        tile_position=(0, 32 * g4))
            ob = outpool.tile([128, 512], F32, tag="ob")
            nc.vector.tensor_scalar_add(ob[:], ps4[:], b3col[:])
            for g4 in range(4):
                nc.sync.dma_start(
                    out=out3[:, sb * SB + 512 * g4: sb * SB + 512 * (g4 + 1)],
                    in_=ob[32 * g4:32 * g4 + OUT, :])


_PROGRAM = None


def _get_program():
    global _PROGRAM
    if _PROGRAM is None:
        _PROGRAM = build_program()
    return _PROGRAM


def _gathered(h16, idx, n_sup):
    """h16[idx] transposed to [n_sup, 128 feats, SB] bf16."""
    g = h16[np.asarray(idx)]                       # [n, 128] bf16
    g = np.ascontiguousarray(g.reshape(n_sup, SB, 128).transpose(0, 2, 1))
    return g


def marshal(inputs, e_sh=E_SH, t_sh=T_SH):
    ins = {k: np.asarray(v) for k, v in inputs.items()}
    mu = ins["mu"].astype(np.float64)
    g128, g126 = _tanh_grid(128, GRID128_LO), _tanh_grid(126, GRID126_LO)
    C128, C126 = _fit_C(g128, mu), _fit_C(g126, mu)

    eW0 = ins["eW0"].astype(np.float32)
    tW0 = ins["tW0"].astype(np.float32)
    eW0p = np.stack([_bf16(eW0[0:128]), _bf16(eW0[128:256]),
                     _bf16(C128.T @ eW0[256:356])])
    t_c3 = np.concatenate([C126.T @ tW0[384:484], tW0[584:586]], axis=0)
    tW0p = np.stack([_bf16(tW0[0:128]), _bf16(tW0[128:256]), _bf16(tW0[256:384]),
                     _bf16(t_c3), _bf16(C128.T @ tW0[484:584])])

    def w2chunks(w):
        w = np.asarray(w, np.float32)
        return np.stack([_bf16(w[0:128]), _bf16(w[128:256])])

    def w3pad(w):
        w = np.asarray(w, np.float32)
        p = np.zeros((2, 128, 32), np.float32)
        p[0, :, :OUT] = w[0:128]
        p[1, :, :OUT] = w[128:256]
        return _bf16(p)

    def b3col(b):
        col = np.zeros((128, 1), np.float32)
        for g in range(4):
            col[32 * g:32 * g + OUT, 0] = np.asarray(b, np.float32)
        return col

    gb128 = (ALPHA * g128).astype(np.float32).reshape(128, 1)
    gb126 = np.zeros((128, 1), np.float32)
    gb126[:126, 0] = (ALPHA * g126).astype(np.float32)

    h16 = _bf16(ins["h"])
    shared = {
        "eW0p": eW0p, "tW0p": tW0p,
        "eW1p": w2chunks(ins["eW1"]), "eW2p": w2chunks(ins["eW2"]),
        "tW1p": w2chunks(ins["tW1"]), "tW2p": w2chunks(ins["tW2"]),
        "eW3p": w3pad(ins["eW3"]), "tW3p": w3pad(ins["tW3"]),
        "gb128": gb128, "gb126": gb126,
        "eb3col": b3col(ins["eb3"]), "tb3col": b3col(ins["tb3"]),
    }
    n_sup_e, n_sup_t = e_sh // SB, t_sh // SB
    in_maps = []
    for c in range(N_CORES):
        esl = slice(c * E_SH, c * E_SH + e_sh)
        tsl = slice(c * T_SH, c * T_SH + t_sh)
        m = dict(shared)
        m["gsrc"] = _gathered(h16, ins["src"][esl], n_sup_e)
        m["gdst"] = _gathered(h16, ins["dst"][esl], n_sup_e)
        m["gts"] = _gathered(h16, ins["t_src"][tsl], n_sup_t)
        m["gtdi"] = _gathered(h16, ins["t_dst_i"][tsl], n_sup_t)
        m["gtdj"] = _gathered(h16, ins["t_dst_j"][tsl], n_sup_t)
        m["enorm"] = np.ascontiguousarray(ins["edge_norm"][esl].astype(np.float32))
        m["nij"] = np.ascontiguousarray(ins["norm_ij"][tsl].astype(np.float32))
        m["nik"] = np.ascontiguousarray(ins["norm_ik"][tsl].astype(np.float32))
        m["cs16"] = np.ascontiguousarray(np.stack(
            [_bf16(ins["cos_ijk"][tsl]), _bf16(ins["sin_ijk"][tsl])]))
        in_maps.append(m)
    return in_maps


def kernel(**inputs):
    nc = _get_program()
    in_maps = marshal(inputs)
    res = run_bass_kernel_spmd(nc, in_maps, core_ids=list(range(N_CORES)))
    e3 = np.concatenate([np.asarray(res.results[i]["out_e3"]) for i in range(N_CORES)], axis=1)
    t3 = np.concatenate([np.asarray(res.results[i]["out_t3"]) for i in range(N_CORES)], axis=1)
    return (np.ascontiguousarray(e3.T.astype(np.float32)),
            np.ascontiguousarray(t3.T.astype(np.float32)))
